# revision 1
# baseline (speedup 1.0000x reference)
"""Trainium2 Bass kernel for CurvatureLoss3D.

Input phi [2,1,192,192,192] f32 -> scalar loss.

Sharding: 8 cores = (batch n in {0,1}) x (depth quarter). Each core gets an
input slab [50,192,192] (depth halo included) and computes per-(h,d)-row
partial sums of pen*mask and mask over its 48 output depth rows. Host trims
edge/overlap rows and finishes the scalar reduction.

On-chip layout: partitions = H, free = (shift s, D, W) where the DMA loads
three H-shifted replicas X3[p,s,d,w] = x(d, h0+p+s, w) via an overlapping
access pattern. A second replica Xc, shifted by +1 in w (SBUF->SBUF DMA),
makes every center-tap (w+1) read 4B-aligned so DVE runs in 2x bf16 mode.
Zero-crossing mask via sign-sum (27 neighbors all same sign <=> |sum| == 27).
Reciprocals via Ln/Exp with exact EPS placement (ACT Reciprocal is banned).
"""

import os
import sys

sys.path.insert(0, "/opt/trn_rl_repo")

import numpy as np

import bass_rust
import concourse.bass as bass
import concourse.tile as tile
from concourse import bacc
from concourse import mybir
from concourse.bass_utils import run_bass_kernel_spmd

F32 = mybir.dt.float32
BF16 = mybir.dt.bfloat16
ALU = mybir.AluOpType
ACTF = mybir.ActivationFunctionType
AX = mybir.AxisListType

EPS = 1e-8
THETA = 0.5 + 1e-8
INV_THETA = 1.0 / THETA

N = 2
DVOL = 192
W = 192
DOUT = 190          # valid conv output extent per axis
D_IN = 50           # input slab depth rows per core
D_OUT_CORE = 48     # output depth rows computed per core
DB = 6              # output d rows per subblock
NSUB = D_OUT_CORE // DB
FD = DB * W         # pointwise free-dim extent
ROW = 3 * W         # one interleaved d-row in X3: shifts s=0,1,2 concatenated
X3W = (DB + 2) * ROW  # data cols in X3
X3PAD = X3W + 2     # +2 pad cols so trailing w+2 reads stay in-bounds
U3E = DB * ROW + 2  # U extent incl. w+1 read at s=2
DB2 = DB + 2        # sign path needs DB+2 d-rows
# (h0, Ph, valid_out_rows)
HBLOCKS = ((0, 128, 126), (126, 64, 64))

# per-core input-slab depth starts; output rows covered = d0..d0+47
CORE_D0 = [0, 48, 96, 142]

_last_results = None  # test harness reads exec time from here


def xo(s, d, w):
    return d * ROW + s * W + w


def _emit(tc, x, band, outp, outc, dbg=None):
    nc = tc.nc
    import contextlib
    import math

    with contextlib.ExitStack() as ctx:
        xpool = ctx.enter_context(tc.tile_pool(name="xin", bufs=3))
        cpool = ctx.enter_context(tc.tile_pool(name="xc", bufs=2))
        mpool = ctx.enter_context(tc.tile_pool(name="main", bufs=2))
        apool = ctx.enter_context(tc.tile_pool(name="acc", bufs=1))
        ppool = ctx.enter_context(tc.tile_pool(name="ps", bufs=2, space="PSUM"))

        accP = apool.tile([128, 2 * D_OUT_CORE], F32, tag="accP", name="accP")
        accC = apool.tile([128, 2 * D_OUT_CORE], F32, tag="accC", name="accC")
        nc.vector.memset(accP[:], 0.0)
        nc.vector.memset(accC[:], 0.0)
        bandt = apool.tile([128, 128], BF16, tag="band", name="bandt")
        nc.sync.dma_start(bandt[:, :], band)

        # bias constants for ACT (only 0.0/1.0 are pre-registered)
        bias_tiles = {}
        for i, bval in enumerate((4.0 * EPS, EPS, math.log(0.25))):
            bt = apool.tile([128, 1], F32, tag=f"bias{i}", name=f"bias{i}")
            nc.gpsimd.memset(bt[:], bval)
            bias_tiles[bval] = bt

        def BIAS(v):
            return bias_tiles[v][:, :]

        def T(tag, fd=FD, dt=BF16):
            return mpool.tile([128, fd], dt, tag=tag, name=tag)

        TT = nc.vector.tensor_tensor
        STT = nc.vector.scalar_tensor_tensor
        TS = nc.vector.tensor_scalar
        TSS = nc.vector.tensor_single_scalar
        ACT = nc.scalar.activation

        for hb, (h0, ph, _hval) in enumerate(HBLOCKS):
            for j in range(NSUB):
                def DUMP(nm, t):
                    if dbg is not None and hb == 0 and j == 0 and nm in dbg:
                        nc.gpsimd.dma_start(dbg[nm], t)
                din0 = DB * j
                Xb = xpool.tile([128, X3PAD], BF16, tag="Xb", name="Xb")
                src = x.copy()
                src.offset = din0 * DVOL * W + h0 * W
                src.ap = bass_rust.VecI64Pair(
                    [[W, ph], [DVOL * W, DB + 2], [1, ROW]]
                )
                nc.sync.dma_start(Xb[0:ph, 0:X3W], src)
                nc.gpsimd.memset(Xb[:, X3W:X3PAD], 1.0)
                # w+1-shifted replica: center taps land on even offsets
                Xc = cpool.tile([128, X3PAD], BF16, tag="Xc", name="Xc")
                nc.sync.dma_start(Xc[0:ph, 0 : X3W + 1], Xb[0:ph, 1 : X3W + 2])
                nc.gpsimd.memset(Xc[:, X3W + 1 : X3PAD], 1.0)

                def xb(s, d, w, n=W):
                    return _view2(Xb, xo(s, d, w), ROW, DB, n)

                def xc(s, d, w, n=W):
                    return _view2(Xc, xo(s, d, w), ROW, DB, n)

                # ---- sign field early (own ACT table slot, feeds PE) ----
                sgn = T("sg", DB2 * 194)  # signs on s=0 block, 194-wide rows
                ACT(_view2(sgn, 0, 194, DB2, 194),
                    _view2(Xb, 0, ROW, DB2, 194), ACTF.Sign)

                # ---- stencil fields (bf16, all reads 4B-aligned) ----
                U3 = T("U3", U3E)  # d-derivative of Xc, all 3 shifts
                TT(U3[:, 0:U3E], Xc[:, 2 * ROW : 2 * ROW + U3E],
                   Xc[:, 0:U3E], ALU.subtract)

                def uoc(s, d, w, n=W):
                    return _view2(U3, xo(s, d, w), ROW, DB, n)

                Vr = T("Vr", DB * 194)  # 2gy on 194-wide rows (w0 base)
                TT(_view2(Vr, 0, 194, DB, 194),
                   _view2(Xb, xo(2, 1, 0), ROW, DB, 194),
                   _view2(Xb, xo(0, 1, 0), ROW, DB, 194), ALU.subtract)

                def vv(w, n=W):
                    return _view2(Vr, w, 194, DB, n)

                Vc = T("Vc")  # 2gy centered (aligned)
                TT(dnv(Vc), xc(2, 1, 0), xc(0, 1, 0), ALU.subtract)

                t1 = T("t1")
                TT(dnv(t1), xc(1, 0, 0), xc(1, 2, 0), ALU.add)
                t2 = T("t2")
                TT(dnv(t2), xb(1, 1, 0), xb(1, 1, 2), ALU.add)
                t3 = T("t3")
                TT(dnv(t3), xc(0, 1, 0), xc(2, 1, 0), ALU.add)
                x2c = T("s4")  # 2*x(d+1,h+1,w+1)
                TS(dnv(x2c), xc(1, 1, 0), 2.0, None, ALU.mult)
                A = T("A")  # hxx
                TT(A[:, :], t1[:, :], x2c[:, :], ALU.subtract)
                C0 = T("C0")  # hzz
                TT(C0[:, :], t2[:, :], x2c[:, :], ALU.subtract)
                B = T("B")  # hyy
                TT(B[:, :], t3[:, :], x2c[:, :], ALU.subtract)
                W1 = T("W1")  # 2gz
                TT(dnv(W1), xb(1, 1, 2), xb(1, 1, 0), ALU.subtract)
                P = T("P")  # 4hxy (aligned via U3c)
                TT(dnv(P), uoc(2, 0, 0), uoc(0, 0, 0), ALU.subtract)
                qa = T("t1")
                TT(dnv(qa), xb(1, 2, 2), xb(1, 0, 2), ALU.subtract)
                qb = T("t2")
                TT(dnv(qb), xb(1, 2, 0), xb(1, 0, 0), ALU.subtract)
                Q = T("Q")  # 4hxz
                TT(Q[:, :], qa[:, :], qb[:, :], ALU.subtract)
                R = T("R")  # 4hyz
                TT(dnv(R), vv(2), vv(0), ALU.subtract)

                # ---- squares (ACT, one table) ----
                U2 = T("U2")
                ACT(dnv(U2), uoc(1, 0, 0), ACTF.Square)
                V2 = T("V2")
                ACT(V2[:, :], Vc[:, :], ACTF.Square)
                W2s = T("W2s")
                ACT(W2s[:, :], W1[:, :], ACTF.Square)

                # ---- S2 = 4|g|^2 and the exact Ln/Exp reciprocal cluster ----
                S2 = T("S2")
                TT(S2[:, :], U2[:, :], V2[:, :], ALU.add)
                TT(S2[:, :], S2[:, :], W2s[:, :], ALU.add)
                DUMP("S2", S2[:, :])
                mg = T("cA")  # mag = sqrt(|grad|^2 + EPS)
                ACT(mg[:, :], S2[:, :], ACTF.Sqrt, scale=0.25, bias=BIAS(EPS))
                mg2 = T("cB")  # mag^2
                TS(mg2[:, :], S2[:, :], 0.25, None, ALU.mult)
                mg3 = T("cE")  # mag^3
                TT(mg3[:, :], mg2[:, :], mg[:, :], ALU.mult)
                LD = T("cC", FD, F32)
                ACT(LD[:, :], mg3[:, :], ACTF.Ln, bias=BIAS(EPS))
                LR = T("cL", FD, F32)
                ACT(LR[:, :], mg[:, :], ACTF.Ln, bias=BIAS(EPS))
                R3q = T("R3")  # 0.25/(mag^3+EPS), bf16
                ACT(R3q[:, :], LD[:, :], ACTF.Exp, scale=-1.0,
                    bias=BIAS(math.log(0.25)))
                R1 = T("R1")  # 1/(mag+EPS), bf16
                ACT(R1[:, :], LR[:, :], ACTF.Exp, scale=-1.0)

                # ---- trace and F = 4*g^T H g (bf16 2x) ----
                trH = T("trH")
                TT(trH[:, :], A[:, :], B[:, :], ALU.add)
                TT(trH[:, :], trH[:, :], C0[:, :], ALU.add)

                # Fc = uvP + uwQ + vwR = u*(vP + wQ) + (vw)*R
                vP = T("s0")
                TT(vP[:, :], Vc[:, :], P[:, :], ALU.mult)
                wQ = T("s1")
                TT(wQ[:, :], W1[:, :], Q[:, :], ALU.mult)
                TT(vP[:, :], vP[:, :], wQ[:, :], ALU.add)
                Fc = T("s2")
                TT(dnv(Fc), uoc(1, 0, 0), _view2(vP, 0, W, DB, W), ALU.mult)
                vw = T("s1")
                TT(vw[:, :], Vc[:, :], W1[:, :], ALU.mult)
                TT(vw[:, :], vw[:, :], R[:, :], ALU.mult)
                TT(Fc[:, :], Fc[:, :], vw[:, :], ALU.add)

                Fd = T("s0")
                TT(Fd[:, :], U2[:, :], A[:, :], ALU.mult)
                F2 = T("s1")
                TT(F2[:, :], V2[:, :], B[:, :], ALU.mult)
                TT(Fd[:, :], Fd[:, :], F2[:, :], ALU.add)
                TT(F2[:, :], W2s[:, :], C0[:, :], ALU.mult)
                TT(Fd[:, :], Fd[:, :], F2[:, :], ALU.add)
                TS(Fc[:, :], Fc[:, :], 0.5, None, ALU.mult)
                Ff = T("s1")  # F = Fd + 0.5*Fc
                TT(Ff[:, :], Fc[:, :], Fd[:, :], ALU.add)

                # ---- curvature glue (bf16) ----
                G = T("s0")
                TT(G[:, :], S2[:, :], trH[:, :], ALU.mult)
                TT(G[:, :], G[:, :], Ff[:, :], ALU.subtract)  # 4*NM
                mc = T("s2")
                TT(mc[:, :], G[:, :], R3q[:, :], ALU.mult)  # mean_c
                qd = T("s3")
                TT(qd[:, :], Ff[:, :], R3q[:, :], ALU.mult)  # quad
                lap = T("s0")
                TT(lap[:, :], trH[:, :], R1[:, :], ALU.mult)
                TT(lap[:, :], lap[:, :], qd[:, :], ALU.subtract)  # gauss
                mc2 = T("s1")
                ACT(mc2[:, :], mc[:, :], ACTF.Square)
                TT(mc2[:, :], mc2[:, :], lap[:, :], ALU.subtract)  # dq
                ad = T("s3")
                ACT(ad[:, :], mc2[:, :], ACTF.Abs)
                sqv = T("s1")
                ACT(sqv[:, :], ad[:, :], ACTF.Sqrt, bias=BIAS(EPS))
                k1 = T("s0")
                TT(k1[:, :], mc[:, :], sqv[:, :], ALU.add)
                k2 = T("s1")
                ACT(k2[:, :], k1[:, :], ACTF.Square, scale=INV_THETA)
                pen = T("s0")
                TS(pen[:, :], k2[:, :], -1.0, 0.0, ALU.add, ALU.max)
                DUMP("pen", pen[:, :])

                # ---- zero-crossing mask: 27-sum of signs via 9 PE matmuls ----
                # h-window via 3-diag band, (d,w)-window via 9 shifted views
                sdp = ppool.tile([128, DB * 256], F32, tag="sdps", name="sdp")
                for dd in range(3):
                    for dw in range(3):
                        for dp in range(0, DB, 2):
                            nc.tensor.matmul(
                                _view2(sdp, dp * 256, 256, 2, W),
                                bandt[:, :],
                                _view2(sgn, (dd + dp) * 194 + dw, 194, 2, W),
                                start=(dd == 0 and dw == 0),
                                stop=(dd == 2 and dw == 2),
                            )
                sd2 = T("t1")
                ACT(dnv(sd2), _view2(sdp, 0, 256, DB, W), ACTF.Square)
                mask = T("t3")
                TSS(mask[:, :], sd2[:, :], 728.5, ALU.is_lt)
                DUMP("mask", mask[:, :])

                # ---- masked penalty + per-d-row reductions over w<190 ----
                penm = T("s0")
                TT(penm[:, :], pen[:, :], mask[:, :], ALU.mult)
                col = hb * D_OUT_CORE + DB * j
                nc.vector.tensor_reduce(
                    accP[:, col : col + DB],
                    _view2(penm, 0, W, DB, DOUT), AX.X, ALU.add,
                )
                nc.vector.tensor_reduce(
                    accC[:, col : col + DB],
                    _view2(mask, 0, W, DB, DOUT), AX.X, ALU.add,
                )

        nc.sync.dma_start(outp, accP[:, :].rearrange("p (b d) -> p b d", b=2))
        nc.sync.dma_start(outc, accC[:, :].rearrange("p (b d) -> p b d", b=2))


def dnv(t, w=0, n=W):
    """dense [d][192] tile view"""
    return _view2(t, w, W, DB, n)


def _install_ntff_hook_shim():
    """Recreate antenv.axon_hooks (absent in this image) so trace=True works."""
    import sys as _sys
    import types
    if "antenv.axon_hooks" in _sys.modules:
        return
    try:
        from trn_agent_boot.trn_boot import _ntff_profile_via_ctypes
        hook = _ntff_profile_via_ctypes("/opt/axon/libaxon_pjrt.so")
    except Exception as e:
        print("ntff shim failed:", e)
        hook = None
    mod = types.ModuleType("antenv.axon_hooks")
    _state = {"hook": hook}
    mod.get_axon_ntff_profile_hook = lambda: _state["hook"]
    mod.set_axon_ntff_profile_hook = lambda h: _state.update(hook=h)
    _sys.modules["antenv.axon_hooks"] = mod
    import antenv
    antenv.axon_hooks = mod


def _view2(t, off, dstep, dcnt, n):
    """AP view of tile t: all partitions, free dims [(dstep, dcnt), (1, n)] at off."""
    ap = t[:, 0:1].copy()
    base = ap.ap.to_list()
    pdim = base[0]
    ap.offset = ap.offset + off
    ap.ap = bass_rust.VecI64Pair([list(pdim), [dstep, dcnt], [1, n]])
    return ap


def _build_nc():
    nc = bacc.Bacc("TRN2", target_bir_lowering=False, debug=False, num_devices=8)
    x = nc.dram_tensor("x", [D_IN, DVOL, W], BF16, kind="ExternalInput")
    band = nc.dram_tensor("band", [128, 128], BF16, kind="ExternalInput")
    outp = nc.dram_tensor("outp", [128, 2, D_OUT_CORE], F32, kind="ExternalOutput")
    outc = nc.dram_tensor("outc", [128, 2, D_OUT_CORE], F32, kind="ExternalOutput")
    with tile.TileContext(nc) as tc:
        _emit(tc, x.ap(), band.ap(), outp.ap(), outc.ap())
    nc.finalize()
    return nc


def kernel(phi):
    global _last_results
    phi = np.asarray(phi)
    assert phi.shape == (N, 1, DVOL, DVOL, W), phi.shape
    nc = _build_nc()
    import ml_dtypes
    phib = phi.astype(ml_dtypes.bfloat16)
    bandm = np.zeros((128, 128), dtype=ml_dtypes.bfloat16)
    for o in range(128):
        for k in range(o, min(o + 3, 128)):
            bandm[k, o] = 1.0
    in_maps = []
    for c in range(8):
        n, q = divmod(c, 4)
        d0 = CORE_D0[q]
        slab = np.ascontiguousarray(phib[n, 0, d0 : d0 + D_IN])
        in_maps.append({"x": slab, "band": bandm})
    trace = bool(int(os.environ.get("KERNEL_TRACE", "0")))
    if trace:
        _install_ntff_hook_shim()
    res = run_bass_kernel_spmd(nc, in_maps, list(range(8)), trace=trace)
    _last_results = res
    tp = 0.0
    tcnt = 0.0
    for c in range(8):
        op = res.results[c]["outp"].astype(np.float64)
        oc = res.results[c]["outc"].astype(np.float64)
        dlo = 2 if (c % 4) == 3 else 0
        for hb, (_h0, _ph, hval) in enumerate(HBLOCKS):
            tp += op[:hval, hb, dlo:].sum()
            tcnt += oc[:hval, hb, dlo:].sum()
    return np.float32(tp / (tcnt + EPS))



# revision 5
# speedup vs baseline: 1.0399x; 1.0399x over previous
"""Trainium2 Bass kernel for CurvatureLoss3D.

Input phi [2,1,192,192,192] f32 -> scalar loss.

Sharding: 8 cores = (batch n in {0,1}) x (depth quarter). Each core gets an
input slab [50,192,192] (depth halo included) and computes per-(h,d)-row
partial sums of pen*mask and mask over its 48 output depth rows. Host trims
edge/overlap rows and finishes the scalar reduction.

On-chip layout: partitions = H, free = (shift s, D, W) where the DMA loads
three H-shifted replicas X3[p,s,d,w] = x(d, h0+p+s, w) via an overlapping
access pattern. A second replica Xc, shifted by +1 in w (SBUF->SBUF DMA),
makes every center-tap (w+1) read 4B-aligned so DVE runs in 2x bf16 mode.
Zero-crossing mask via sign-sum (27 neighbors all same sign <=> |sum| == 27).
Reciprocals via Ln/Exp with exact EPS placement (ACT Reciprocal is banned).
"""

import os
import sys

sys.path.insert(0, "/opt/trn_rl_repo")

import numpy as np

import bass_rust
import concourse.bass as bass
import concourse.tile as tile
from concourse import bacc
from concourse import mybir
from concourse.bass_utils import run_bass_kernel_spmd

F32 = mybir.dt.float32
BF16 = mybir.dt.bfloat16
ALU = mybir.AluOpType
ACTF = mybir.ActivationFunctionType
AX = mybir.AxisListType

EPS = 1e-8
THETA = 0.5 + 1e-8
INV_THETA = 1.0 / THETA

N = 2
DVOL = 192
W = 192
DOUT = 190          # valid conv output extent per axis
D_IN = 50           # input slab depth rows per core
D_OUT_CORE = 48     # output depth rows computed per core
DB = 6              # output d rows per subblock
NSUB = D_OUT_CORE // DB
FD = DB * W         # pointwise free-dim extent
ROW = 3 * W         # one interleaved d-row in X3: shifts s=0,1,2 concatenated
X3W = (DB + 2) * ROW  # data cols in X3
X3PAD = X3W + 2     # +2 pad cols so trailing w+2 reads stay in-bounds
U3E = DB * ROW + 2  # U extent incl. w+1 read at s=2
DB2 = DB + 2        # sign path needs DB+2 d-rows
# (h0, Ph, valid_out_rows)
HBLOCKS = ((0, 128, 126), (126, 64, 64))

# per-core input-slab depth starts; output rows covered = d0..d0+47
CORE_D0 = [0, 48, 96, 142]

_last_results = None  # test harness reads exec time from here


def xo(s, d, w):
    return d * ROW + s * W + w


def _emit(tc, x, band, outp, outc, dbg=None):
    nc = tc.nc
    import contextlib
    import math

    with contextlib.ExitStack() as ctx:
        xpool = ctx.enter_context(tc.tile_pool(name="xin", bufs=3))
        cpool = ctx.enter_context(tc.tile_pool(name="xc", bufs=2))
        mpool = ctx.enter_context(tc.tile_pool(name="main", bufs=2))
        apool = ctx.enter_context(tc.tile_pool(name="acc", bufs=1))
        ppool = ctx.enter_context(tc.tile_pool(name="ps", bufs=2, space="PSUM"))

        accP = apool.tile([128, 2 * D_OUT_CORE], F32, tag="accP", name="accP")
        accC = apool.tile([128, 2 * D_OUT_CORE], F32, tag="accC", name="accC")
        nc.vector.memset(accP[:], 0.0)
        nc.vector.memset(accC[:], 0.0)
        bandt = apool.tile([128, 128], BF16, tag="band", name="bandt")
        nc.sync.dma_start(bandt[:, :], band)

        # bias constants for ACT (only 0.0/1.0 are pre-registered)
        bias_tiles = {}
        for i, bval in enumerate(
            (4.0 * EPS, EPS, math.log(0.5), math.log(4.0 / THETA**2))
        ):
            bt = apool.tile([128, 1], F32, tag=f"bias{i}", name=f"bias{i}")
            nc.gpsimd.memset(bt[:], bval)
            bias_tiles[bval] = bt

        def BIAS(v):
            return bias_tiles[v][:, :]

        def T(tag, fd=FD, dt=BF16):
            return mpool.tile([128, fd], dt, tag=tag, name=tag)

        TT = nc.vector.tensor_tensor
        STT = nc.vector.scalar_tensor_tensor
        TS = nc.vector.tensor_scalar
        TSS = nc.vector.tensor_single_scalar
        ACT = nc.scalar.activation

        for hb, (h0, ph, _hval) in enumerate(HBLOCKS):
            for j in range(NSUB):
                def DUMP(nm, t):
                    if dbg is not None and hb == 0 and j == 0 and nm in dbg:
                        nc.gpsimd.dma_start(dbg[nm], t)
                din0 = DB * j
                Xb = xpool.tile([128, X3PAD], BF16, tag="Xb", name="Xb")
                src = x.copy()
                src.offset = din0 * DVOL * W + h0 * W
                src.ap = bass_rust.VecI64Pair(
                    [[W, ph], [DVOL * W, DB + 2], [1, ROW]]
                )
                nc.sync.dma_start(Xb[0:ph, 0:X3W], src)
                nc.gpsimd.memset(Xb[:, X3W:X3PAD], 1.0)
                # w+1-shifted replica: center taps land on even offsets
                Xc = cpool.tile([128, X3PAD], BF16, tag="Xc", name="Xc")
                nc.sync.dma_start(Xc[0:ph, 0 : X3W + 1], Xb[0:ph, 1 : X3W + 2])
                nc.gpsimd.memset(Xc[:, X3W + 1 : X3PAD], 1.0)

                def xb(s, d, w, n=W):
                    return _view2(Xb, xo(s, d, w), ROW, DB, n)

                def xc(s, d, w, n=W):
                    return _view2(Xc, xo(s, d, w), ROW, DB, n)

                # ---- sign field early (own ACT table slot, feeds PE) ----
                sgn = T("sg", DB2 * 194)  # signs on s=0 block, 194-wide rows
                ACT(_view2(sgn, 0, 194, DB2, 194),
                    _view2(Xb, 0, ROW, DB2, 194), ACTF.Sign)

                # ---- stencil fields (bf16, all reads 4B-aligned) ----
                U3 = T("U3", U3E)  # d-derivative of Xc, all 3 shifts
                TT(U3[:, 0:U3E], Xc[:, 2 * ROW : 2 * ROW + U3E],
                   Xc[:, 0:U3E], ALU.subtract)

                def uoc(s, d, w, n=W):
                    return _view2(U3, xo(s, d, w), ROW, DB, n)

                Vr = T("Vr", DB * 194)  # 2gy on 194-wide rows (w0 base)
                TT(_view2(Vr, 0, 194, DB, 194),
                   _view2(Xb, xo(2, 1, 0), ROW, DB, 194),
                   _view2(Xb, xo(0, 1, 0), ROW, DB, 194), ALU.subtract)

                def vv(w, n=W):
                    return _view2(Vr, w, 194, DB, n)

                Vc = T("Vc")  # 2gy centered (aligned)
                TT(dnv(Vc), xc(2, 1, 0), xc(0, 1, 0), ALU.subtract)

                t1 = T("t1")
                TT(dnv(t1), xc(1, 0, 0), xc(1, 2, 0), ALU.add)
                t2 = T("t2")
                TT(dnv(t2), xb(1, 1, 0), xb(1, 1, 2), ALU.add)
                t3 = T("t3")
                TT(dnv(t3), xc(0, 1, 0), xc(2, 1, 0), ALU.add)
                x2c = T("s4")  # 2*x(d+1,h+1,w+1)
                TS(dnv(x2c), xc(1, 1, 0), 2.0, None, ALU.mult)
                A = T("A")  # hxx
                TT(A[:, :], t1[:, :], x2c[:, :], ALU.subtract)
                C0 = T("C0")  # hzz
                TT(C0[:, :], t2[:, :], x2c[:, :], ALU.subtract)
                B = T("B")  # hyy
                TT(B[:, :], t3[:, :], x2c[:, :], ALU.subtract)
                W1 = T("W1")  # 2gz
                TT(dnv(W1), xb(1, 1, 2), xb(1, 1, 0), ALU.subtract)
                P = T("P")  # 4hxy (aligned via U3c)
                TT(dnv(P), uoc(2, 0, 0), uoc(0, 0, 0), ALU.subtract)
                # 4hxz = u[w+2]-u[w]: odd-offset views of U3 (1x DVE mode but
                # one op instead of three)
                Q = T("Q")  # 4hxz
                TT(
                    Q[:, :],
                    _view2(U3, xo(1, 0, 0) + 1, ROW, DB, W),
                    _view2(U3, xo(1, 0, 0) - 1, ROW, DB, W),
                    ALU.subtract,
                )
                R = T("R")  # 4hyz
                TT(dnv(R), vv(2), vv(0), ALU.subtract)

                # ---- squares (ACT, one table) ----
                U2 = T("U2")
                ACT(dnv(U2), uoc(1, 0, 0), ACTF.Square)
                V2 = T("V2")
                ACT(V2[:, :], Vc[:, :], ACTF.Square)
                W2s = T("W2s")
                ACT(W2s[:, :], W1[:, :], ACTF.Square)

                # ---- S2 = 4|g|^2; scaled-curvature algebra ----
                # With S2' = S2+4EPS, M3 = 0.5*S2'^1.5 (= 4(mag^3+eps')):
                #   mean_c = G/M3, quad = Ff/M3, gauss = (S2'*trH - Ff)/M3
                #   dq*M3^2 = G^2 - M3*G =: D   (to O(EPS*trH))
                #   k1 = (G + sqrt(|D|+eps))/M3
                #   (k1/theta)^2 = (G+sqv)^2 * r,  r = (4/theta^2)*exp(-3 ln S2')
                # Only Ln/Exp/Square/Abs/Sign ACTs -> a single ACT table.
                S2 = T("S2")
                TT(S2[:, :], U2[:, :], V2[:, :], ALU.add)
                TT(S2[:, :], S2[:, :], W2s[:, :], ALU.add)
                DUMP("S2", S2[:, :])
                # clamp keeps mag^3 >> EPS so the scaled algebra stays exact;
                # only voxels with |grad phi| < 0.005 (a handful, ref pen ~1e5
                # of a ~3e9 total) are perturbed
                S2m = T("cB")
                TS(S2m[:, :], S2[:, :], 1e-4, None, ALU.max)
                Ltile = T("cC", FD, F32)  # ln(max(S2,1e-4)+4EPS)
                ACT(Ltile[:, :], S2m[:, :], ACTF.Ln, bias=BIAS(4.0 * EPS))
                m3h = T("cA")  # M3 = 0.5*S2'^1.5
                ACT(m3h[:, :], Ltile[:, :], ACTF.Exp, scale=1.5,
                    bias=BIAS(math.log(0.5)))
                rr = T("R3")  # 1/(theta^2*M3^2) = (4/theta^2)*S2'^-3
                ACT(rr[:, :], Ltile[:, :], ACTF.Exp, scale=-3.0,
                    bias=BIAS(math.log(4.0 / THETA**2)))

                # ---- trace and F = 4*g^T H g (bf16 2x) ----
                trH = T("trH")
                TT(trH[:, :], A[:, :], B[:, :], ALU.add)
                TT(trH[:, :], trH[:, :], C0[:, :], ALU.add)

                # Fc = uvP + uwQ + vwR = u*(vP + wQ) + (vw)*R
                vP = T("s0")
                TT(vP[:, :], Vc[:, :], P[:, :], ALU.mult)
                wQ = T("s1")
                TT(wQ[:, :], W1[:, :], Q[:, :], ALU.mult)
                TT(vP[:, :], vP[:, :], wQ[:, :], ALU.add)
                Fc = T("s2")
                TT(dnv(Fc), uoc(1, 0, 0), _view2(vP, 0, W, DB, W), ALU.mult)
                vw = T("s1")
                TT(vw[:, :], Vc[:, :], W1[:, :], ALU.mult)
                TT(vw[:, :], vw[:, :], R[:, :], ALU.mult)
                TT(Fc[:, :], Fc[:, :], vw[:, :], ALU.add)

                Fd = T("s0")
                TT(Fd[:, :], U2[:, :], A[:, :], ALU.mult)
                F2 = T("s1")
                TT(F2[:, :], V2[:, :], B[:, :], ALU.mult)
                TT(Fd[:, :], Fd[:, :], F2[:, :], ALU.add)
                TT(F2[:, :], W2s[:, :], C0[:, :], ALU.mult)
                TT(Fd[:, :], Fd[:, :], F2[:, :], ALU.add)
                Ff = T("s1")  # F = Fd + 0.5*Fc
                STT(Ff[:, :], Fc[:, :], 0.5, Fd[:, :], ALU.mult, ALU.add)

                # ---- curvature glue (bf16) ----
                G = T("s0")
                TT(G[:, :], S2[:, :], trH[:, :], ALU.mult)
                TT(G[:, :], G[:, :], Ff[:, :], ALU.subtract)  # 4*NM
                Gsq = T("s2")
                ACT(Gsq[:, :], G[:, :], ACTF.Square)
                tm = T("s3")
                TT(tm[:, :], m3h[:, :], G[:, :], ALU.mult)  # M3*G
                Dq = T("s1")
                TT(Dq[:, :], Gsq[:, :], tm[:, :], ALU.subtract)  # dq*M3^2
                ad = T("s3")
                ACT(ad[:, :], Dq[:, :], ACTF.Abs)
                lnD = T("cL", FD, F32)
                ACT(lnD[:, :], ad[:, :], ACTF.Ln, bias=BIAS(EPS))
                sqv = T("s1")
                ACT(sqv[:, :], lnD[:, :], ACTF.Exp, scale=0.5)
                num = T("s0")
                TT(num[:, :], G[:, :], sqv[:, :], ALU.add)  # k1*M3
                numsq = T("s2")
                ACT(numsq[:, :], num[:, :], ACTF.Square)
                k2 = T("s1")
                TT(k2[:, :], numsq[:, :], rr[:, :], ALU.mult)
                pen = T("s0")
                TS(pen[:, :], k2[:, :], -1.0, 0.0, ALU.add, ALU.max)
                DUMP("pen", pen[:, :])

                # ---- zero-crossing mask: 27-sum of signs via 9 PE matmuls ----
                # h-window via 3-diag band, (d,w)-window via 9 shifted views
                sdp = ppool.tile([128, DB * 256], F32, tag="sdps", name="sdp")
                for dd in range(3):
                    for dw in range(3):
                        for dp in range(0, DB, 2):
                            nc.tensor.matmul(
                                _view2(sdp, dp * 256, 256, 2, W),
                                bandt[:, :],
                                _view2(sgn, (dd + dp) * 194 + dw, 194, 2, W),
                                start=(dd == 0 and dw == 0),
                                stop=(dd == 2 and dw == 2),
                            )
                sd2 = T("t1")
                ACT(dnv(sd2), _view2(sdp, 0, 256, DB, W), ACTF.Square)
                mask = T("t3")
                TSS(mask[:, :], sd2[:, :], 728.5, ALU.is_lt)
                DUMP("mask", mask[:, :])

                # ---- masked penalty + per-d-row reductions over w<190 ----
                penm = T("s0")
                TT(penm[:, :], pen[:, :], mask[:, :], ALU.mult)
                col = hb * D_OUT_CORE + DB * j
                nc.vector.tensor_reduce(
                    accP[:, col : col + DB],
                    _view2(penm, 0, W, DB, DOUT), AX.X, ALU.add,
                )
                nc.vector.tensor_reduce(
                    accC[:, col : col + DB],
                    _view2(mask, 0, W, DB, DOUT), AX.X, ALU.add,
                )

        nc.sync.dma_start(outp, accP[:, :].rearrange("p (b d) -> p b d", b=2))
        nc.sync.dma_start(outc, accC[:, :].rearrange("p (b d) -> p b d", b=2))


def dnv(t, w=0, n=W):
    """dense [d][192] tile view"""
    return _view2(t, w, W, DB, n)


def _install_ntff_hook_shim():
    """Recreate antenv.axon_hooks (absent in this image) so trace=True works."""
    import sys as _sys
    import types
    if "antenv.axon_hooks" in _sys.modules:
        return
    try:
        from trn_agent_boot.trn_boot import _ntff_profile_via_ctypes
        hook = _ntff_profile_via_ctypes("/opt/axon/libaxon_pjrt.so")
    except Exception as e:
        print("ntff shim failed:", e)
        hook = None
    mod = types.ModuleType("antenv.axon_hooks")
    _state = {"hook": hook}
    mod.get_axon_ntff_profile_hook = lambda: _state["hook"]
    mod.set_axon_ntff_profile_hook = lambda h: _state.update(hook=h)
    _sys.modules["antenv.axon_hooks"] = mod
    import antenv
    antenv.axon_hooks = mod


def _view2(t, off, dstep, dcnt, n):
    """AP view of tile t: all partitions, free dims [(dstep, dcnt), (1, n)] at off."""
    ap = t[:, 0:1].copy()
    base = ap.ap.to_list()
    pdim = base[0]
    ap.offset = ap.offset + off
    ap.ap = bass_rust.VecI64Pair([list(pdim), [dstep, dcnt], [1, n]])
    return ap


def _build_nc():
    nc = bacc.Bacc("TRN2", target_bir_lowering=False, debug=False, num_devices=8)
    x = nc.dram_tensor("x", [D_IN, DVOL, W], BF16, kind="ExternalInput")
    band = nc.dram_tensor("band", [128, 128], BF16, kind="ExternalInput")
    outp = nc.dram_tensor("outp", [128, 2, D_OUT_CORE], F32, kind="ExternalOutput")
    outc = nc.dram_tensor("outc", [128, 2, D_OUT_CORE], F32, kind="ExternalOutput")
    with tile.TileContext(nc) as tc:
        _emit(tc, x.ap(), band.ap(), outp.ap(), outc.ap())
    nc.finalize()
    return nc


def kernel(phi):
    global _last_results
    phi = np.asarray(phi)
    assert phi.shape == (N, 1, DVOL, DVOL, W), phi.shape
    nc = _build_nc()
    import ml_dtypes
    phib = phi.astype(ml_dtypes.bfloat16)
    bandm = np.zeros((128, 128), dtype=ml_dtypes.bfloat16)
    for o in range(128):
        for k in range(o, min(o + 3, 128)):
            bandm[k, o] = 1.0
    in_maps = []
    for c in range(8):
        n, q = divmod(c, 4)
        d0 = CORE_D0[q]
        slab = np.ascontiguousarray(phib[n, 0, d0 : d0 + D_IN])
        in_maps.append({"x": slab, "band": bandm})
    trace = bool(int(os.environ.get("KERNEL_TRACE", "0")))
    if trace:
        _install_ntff_hook_shim()
    res = run_bass_kernel_spmd(nc, in_maps, list(range(8)), trace=trace)
    _last_results = res
    tp = 0.0
    tcnt = 0.0
    for c in range(8):
        op = res.results[c]["outp"].astype(np.float64)
        oc = res.results[c]["outc"].astype(np.float64)
        dlo = 2 if (c % 4) == 3 else 0
        for hb, (_h0, _ph, hval) in enumerate(HBLOCKS):
            tp += op[:hval, hb, dlo:].sum()
            tcnt += oc[:hval, hb, dlo:].sum()
    return np.float32(tp / (tcnt + EPS))

